# revision 13
# baseline (speedup 1.0000x reference)
"""Causal self-attention Bass/Tile kernel for TRN2, data-parallel over 8 NeuronCores.

Shapes (hardcoded): x [16, 1024, 1024] f32, W_attn [1024, 3072], b_attn [3072],
W_proj [1024, 1024], b_proj [1024].  16 heads, head dim 64.
Each core processes 2 batch elements end-to-end; no collectives.

Design notes (v2):
  - x is uploaded pre-transposed per batch ([C, T] tiles) so no PE transposes.
  - All weights resident in SBUF, loaded once (W_attn 48KB/part, W_proj 16KB/part).
  - q^T,k^T per head pair via (W tile).T @ x^T; v natural via (x^T tile).T @ W_v.
  - scores^T = k^T.T @ q^T, K=64 row-packed pairs (tile_position) run concurrently.
  - AV: vext.T @ P^T col-packed with a ones[128,64] stationary at tile_position
    (0,64) so PSUM partitions 64..127 accumulate the softmax denominators;
    normalization = reciprocal + fused multiply-evict, no partition broadcast.
  - PE warm-up matmuls during the initial DMA wait (HAM clock gate).
  - qk of the next head pair / v / proj matmuls are interleaved as filler between
    score pairs to cover exp latency (PE queue is strict FIFO).
"""
import sys

sys.path.insert(0, "/opt/trn_rl_repo")

from contextlib import ExitStack

import numpy as np

import concourse.bass as bass
import concourse.mybir as mybir
import concourse.tile as tile
from concourse import bacc
from concourse.bass_utils import run_bass_kernel_spmd
from concourse.masks import make_upper_triangular

F32 = mybir.dt.float32
BF16 = mybir.dt.bfloat16
EXP = mybir.ActivationFunctionType.Exp

N_CORES = 8
B, T, C = 16, 1024, 1024
H, DH = 16, 64
BL = B // N_CORES          # batches per core
TT = T // 128              # token tiles (8)
KO = C // 128              # contraction chunks (8)
NQ = T // 512              # 512-wide token chunks (2)
SCALE = 1.0 / 8.0          # 1/sqrt(64)


def _emit(nc, tc, xT_d, wa_d, wp_d, bqk_d, bv_d, bpj_d, out_d, DBG=None):
    with ExitStack() as ctx:
        const = ctx.enter_context(tc.tile_pool(name="const", bufs=1))
        w_pool = ctx.enter_context(tc.tile_pool(name="w", bufs=1))
        xT_pool = ctx.enter_context(tc.tile_pool(name="xT", bufs=2))
        vext_pool = ctx.enter_context(tc.tile_pool(name="vext", bufs=2))
        qk_pool = ctx.enter_context(tc.tile_pool(name="qk", bufs=3))
        pt_pool = ctx.enter_context(tc.tile_pool(name="pt", bufs=6))
        yT_pool = ctx.enter_context(tc.tile_pool(name="yT", bufs=2))
        rec_pool = ctx.enter_context(tc.tile_pool(name="rec", bufs=2))
        out_pool = ctx.enter_context(tc.tile_pool(name="out", bufs=4))
        psQ = ctx.enter_context(tc.tile_pool(name="psQ", bufs=2, space="PSUM"))
        psS = ctx.enter_context(tc.tile_pool(name="psS", bufs=2, space="PSUM"))
        psAV = ctx.enter_context(tc.tile_pool(name="psAV", bufs=1, space="PSUM"))

        # ---- constants ----
        zmask = const.tile([128, 128], F32)
        make_upper_triangular(nc, zmask, val=1.0, diag=True)
        ones64 = const.tile([128, 64], BF16)
        nc.vector.memset(ones64, 1.0)
        warm = const.tile([128, 512], BF16)
        nc.vector.memset(warm, 0.0)
        b_qk = const.tile([128, 16], F32)
        nc.sync.dma_start(b_qk, bqk_d)
        bv_b = const.tile([128, C], F32)
        nc.sync.dma_start(bv_b, bv_d[0:1, :].to_broadcast((128, C)))
        bpj_b = const.tile([128, C], F32)
        nc.sync.dma_start(bpj_b, bpj_d[0:1, :].to_broadcast((128, C)))

        # ---- resident weights (split DMAs across queues) ----
        wa_sb = w_pool.tile([128, KO, 3 * C], BF16, tag="wa", name="wa")
        for k in range(KO):
            nc.sync.dma_start(wa_sb[:, k, :], wa_d[:, k, :])
        wp_sb = w_pool.tile([128, KO, C], BF16, tag="wp", name="wp")
        for k in range(KO):
            nc.sync.dma_start(wp_sb[:, k, :], wp_d[:, k, :])

        # ---- x^T tiles ----
        xT = {}
        for b in range(BL):
            xT[b] = xT_pool.tile([128, KO, T], BF16, tag="xT", name=f"xT{b}")
            for k in range(KO):
                nc.sync.dma_start(xT[b][:, k, :], xT_d[b, k, :, :])

        # ---- PE warm-up during DMA wait (HAM clock gate) ----
        for i in range(10):
            wps = psQ.tile([128, 512], F32, tag="q", name=f"warm{i}")
            nc.tensor.matmul(wps, warm[:, 0:128], warm, start=True, stop=True)

        vext = {}
        yT = {}
        qk = {}

        # ---------- thunk builders (each thunk = one PE matmul + attached evicts) ----------

        def v_thunks(b):
            """v for all heads of batch b -> vext[b]; 128 mms in 16 groups."""
            vext[b] = vext_pool.tile([128, TT, H, DH], BF16, tag="vext", name=f"vext{b}")
            thunks = []
            for nn in range(NQ):
                for m in range(TT):
                    ps = psQ.tile([128, 512], F32, tag="q", name=f"vps{b}_{nn}_{m}")
                    for k in range(KO):
                        def th(nn=nn, m=m, ps=ps, k=k):
                            nc.tensor.matmul(
                                ps,
                                xT[b][:, k, m * 128 : (m + 1) * 128],
                                wa_sb[:, k, 2 * C + nn * 512 : 2 * C + (nn + 1) * 512],
                                start=(k == 0),
                                stop=(k == KO - 1),
                            )
                            if k == KO - 1:
                                nc.vector.tensor_add(
                                    vext[b][:, m, nn * 8 : (nn + 1) * 8, :],
                                    ps.rearrange("p (h d) -> p h d", d=DH),
                                    bv_b[:, nn * 512 : (nn + 1) * 512].rearrange(
                                        "p (h d) -> p h d", d=DH
                                    ),
                                )
                        thunks.append(th)
            return thunks

        def qk_thunks(b, hp):
            """q^T,k^T for head pair hp of batch b; 32 mms in 4 groups."""
            qk[(b, hp)] = qk_pool.tile([128, 2, T], BF16, tag="qk", name=f"qk{b}_{hp}")
            thunks = []
            for which, mt in ((0, hp), (1, 8 + hp)):
                for nn in range(NQ):
                    ps = psQ.tile([128, 512], F32, tag="q", name=f"qkps{b}_{mt}_{nn}")
                    for k in range(KO):
                        def th(which=which, mt=mt, nn=nn, ps=ps, k=k):
                            nc.tensor.matmul(
                                ps,
                                wa_sb[:, k, mt * 128 : (mt + 1) * 128],
                                xT[b][:, k, nn * 512 : (nn + 1) * 512],
                                start=(k == 0),
                                stop=(k == KO - 1),
                            )
                            if k == KO - 1:
                                nc.vector.tensor_add(
                                    qk[(b, hp)][:, which, nn * 512 : (nn + 1) * 512],
                                    ps,
                                    b_qk[:, mt : mt + 1].to_broadcast((128, 512)),
                                )
                        thunks.append(th)
            return thunks

        def proj_thunks(b):
            """out = y @ W_proj + b_proj for batch b; 128 mms in 16 groups."""
            thunks = []
            for nn in range(NQ):
                for m in range(TT):
                    ps = psQ.tile([128, 512], F32, tag="q", name=f"pps{b}_{nn}_{m}")
                    for k in range(KO):
                        def th(nn=nn, m=m, ps=ps, k=k):
                            nc.tensor.matmul(
                                ps,
                                yT[b][:, k, m * 128 : (m + 1) * 128],
                                wp_sb[:, k, nn * 512 : (nn + 1) * 512],
                                start=(k == 0),
                                stop=(k == KO - 1),
                            )
                            if k == KO - 1:
                                osb = out_pool.tile(
                                    [128, 512], F32, tag="o", name=f"os{b}_{nn}_{m}"
                                )
                                nc.vector.tensor_add(
                                    osb, ps, bpj_b[:, nn * 512 : (nn + 1) * 512]
                                )
                                nc.sync.dma_start(
                                    out_d[
                                        b,
                                        m * 128 : (m + 1) * 128,
                                        nn * 512 : (nn + 1) * 512,
                                    ],
                                    osb,
                                )
                        thunks.append(th)
            return thunks

        # ---------- attention inner loop for one head pair ----------

        def hp_loop(b, hp, fill):
            qk_t = qk[(b, hp)]
            n_slots = 12
            per_slot = max(1, -(-len(fill) // n_slots))
            fi = 0

            def emit_fill(n):
                nonlocal fi
                for _ in range(n):
                    if fi < len(fill):
                        fill[fi]()
                        fi += 1

            for qc in range(NQ):
                nkt = 4 * qc + 4
                Y0 = psAV.tile([128, 512], F32, tag="y0", name=f"y0_{b}_{hp}_{qc}")
                Y1 = psAV.tile([128, 512], F32, tag="y1", name=f"y1_{b}_{hp}_{qc}")
                D0 = psAV.tile([128, 512], F32, tag="d0", name=f"d0_{b}_{hp}_{qc}")
                D1 = psAV.tile([128, 512], F32, tag="d1", name=f"d1_{b}_{hp}_{qc}")
                h0, h1 = 2 * hp, 2 * hp + 1

                def av_slots(kt, pts, last):
                    # one PSUM bank per array position: concurrent pairs always
                    # target different banks (same-bank concurrent drain corrupts)
                    pst = 0 if kt - 4 * qc < 0 else (kt - 4 * qc) * 128
                    pt0, pt1 = pts
                    first = kt == 0
                    # slot A: V(h0)@(0,0)->Y0[0:64]  ||  ones@(0,64)->D1[64:128]
                    nc.tensor.matmul(
                        Y0[0:64, pst:512],
                        vext[b][:, kt, h0, :],
                        pt0[:, pst:512],
                        start=first,
                        stop=last,
                        tile_position=(0, 0),
                    )
                    nc.tensor.matmul(
                        D1[64:128, pst:512],
                        ones64,
                        pt1[:, pst:512],
                        start=first,
                        stop=last,
                        tile_position=(0, 64),
                    )
                    # slot B: ones@(0,0)->D0[0:64]  ||  V(h1)@(0,64)->Y1[64:128]
                    nc.tensor.matmul(
                        D0[0:64, pst:512],
                        ones64,
                        pt0[:, pst:512],
                        start=first,
                        stop=last,
                        tile_position=(0, 0),
                    )
                    nc.tensor.matmul(
                        Y1[64:128, pst:512],
                        vext[b][:, kt, h1, :],
                        pt1[:, pst:512],
                        start=first,
                        stop=last,
                        tile_position=(0, 64),
                    )

                prev = None
                for kt in range(nkt):
                    j = kt - 4 * qc
                    st = 0 if j < 0 else j * 128
                    # score pair (row-packed, concurrent, different banks)
                    sps = {}
                    for h2 in range(2):
                        sp = psS.tile(
                            [128, 512], F32, tag="sc", name=f"sc{b}_{hp}_{qc}_{kt}_{h2}"
                        )
                        nc.tensor.matmul(
                            sp[:, st:512],
                            qk_t[64 * h2 : 64 * h2 + 64, 1, kt * 128 : (kt + 1) * 128],
                            qk_t[
                                64 * h2 : 64 * h2 + 64,
                                0,
                                qc * 512 + st : (qc + 1) * 512,
                            ],
                            start=True,
                            stop=True,
                            tile_position=(64 * h2, 0),
                        )
                        sps[h2] = sp
                    emit_fill(per_slot)
                    # AV for the previous kt (exp latency covered by filler above)
                    if prev is not None:
                        av_slots(prev[0], prev[1], last=False)
                    # exp (+ causal mask on the diagonal block)
                    cpts = []
                    for h2 in range(2):
                        pt = pt_pool.tile(
                            [128, 512], BF16, tag="pt", name=f"pt{b}_{hp}_{qc}_{kt}_{h2}"
                        )
                        nc.scalar.activation(
                            pt[:, st:512], sps[h2][:, st:512], EXP, scale=SCALE
                        )
                        if j >= 0:
                            nc.vector.tensor_mul(
                                pt[:, st : st + 128], pt[:, st : st + 128], zmask
                            )
                        cpts.append(pt)
                    prev = (kt, cpts)
                # AV for the last kt
                av_slots(prev[0], prev[1], last=True)
                # normalize + evict into yT (partition-aligned per head)
                rec = rec_pool.tile([128, 512], F32, tag="rec", name=f"rec{b}_{hp}_{qc}")
                nc.vector.reciprocal(rec[0:64, :], D0[0:64, :])
                nc.vector.reciprocal(rec[64:128, :], D1[64:128, :])
                nc.vector.tensor_mul(
                    yT[b][0:64, hp, qc * 512 : (qc + 1) * 512], Y0[0:64, :], rec[0:64, :]
                )
                nc.vector.tensor_mul(
                    yT[b][64:128, hp, qc * 512 : (qc + 1) * 512],
                    Y1[64:128, :],
                    rec[64:128, :],
                )
            # drain remaining filler
            emit_fill(len(fill))
            if DBG is not None and b == 0:
                nc.sync.dma_start(DBG["qk"][hp], qk_t)

        # ---------- schedule ----------
        for b in range(BL):
            yT[b] = yT_pool.tile([128, KO, T], BF16, tag="yT", name=f"yT{b}")

        for th in v_thunks(0):
            th()
        for th in qk_thunks(0, 0):
            th()

        v1 = v_thunks(1)
        for hp in range(TT):
            fill = []
            if hp < 7:
                fill += qk_thunks(0, hp + 1)
            if hp >= 4:
                fill += v1[(hp - 4) * 32 : (hp - 3) * 32]
            if hp == 7:
                fill += qk_thunks(1, 0)
            hp_loop(0, hp, fill)

        p0 = proj_thunks(0)
        for hp in range(TT):
            fill = []
            if hp < 7:
                fill += qk_thunks(1, hp + 1)
            fill += p0[hp * 16 : (hp + 1) * 16]
            hp_loop(1, hp, fill)

        for th in proj_thunks(1):
            th()

        if DBG is not None:
            nc.sync.dma_start(DBG["vext"], vext[0])
            nc.sync.dma_start(DBG["yT"], yT[0])


_CACHE = {}


def _build(debug=False):
    key = "nc_dbg" if debug else "nc"
    if key in _CACHE:
        return _CACHE[key]
    nc = bacc.Bacc("TRN2", target_bir_lowering=False, debug=False)
    xT_d = nc.dram_tensor("xT", [BL, KO, 128, T], BF16, kind="ExternalInput").ap()
    wa_d = nc.dram_tensor("wa", [128, KO, 3 * C], BF16, kind="ExternalInput").ap()
    wp_d = nc.dram_tensor("wp", [128, KO, C], BF16, kind="ExternalInput").ap()
    bqk_d = nc.dram_tensor("bqk", [128, 16], F32, kind="ExternalInput").ap()
    bv_d = nc.dram_tensor("bv", [1, C], F32, kind="ExternalInput").ap()
    bpj_d = nc.dram_tensor("bpj", [1, C], F32, kind="ExternalInput").ap()
    out_d = nc.dram_tensor("out", [BL, T, C], F32, kind="ExternalOutput").ap()
    DBG = None
    if debug:
        DBG = {
            "qk": [
                nc.dram_tensor(f"dqk{hp}", [128, 2, T], BF16, kind="ExternalOutput").ap()
                for hp in range(KO)
            ],
            "vext": nc.dram_tensor("dvext", [128, TT, H, DH], BF16, kind="ExternalOutput").ap(),
            "yT": nc.dram_tensor("dyT", [128, KO, T], BF16, kind="ExternalOutput").ap(),
        }
    with tile.TileContext(nc) as tc:
        _emit(nc, tc, xT_d, wa_d, wp_d, bqk_d, bv_d, bpj_d, out_d, DBG=DBG)
    nc.compile()
    _CACHE[key] = nc
    return nc


def kernel(x, W_attn, b_attn, W_proj, b_proj, _trace=False):
    nc = _build()
    import ml_dtypes

    bf = ml_dtypes.bfloat16
    x = np.asarray(x, dtype=np.float32)
    # per-batch transpose to [C, T], tiled [KO, 128, T]
    xT = np.ascontiguousarray(np.transpose(x, (0, 2, 1))).reshape(B, KO, 128, T).astype(bf)
    wa = np.ascontiguousarray(
        np.asarray(W_attn, dtype=np.float32).reshape(KO, 128, 3 * C).transpose(1, 0, 2)
    ).astype(bf)
    wp = np.ascontiguousarray(
        np.asarray(W_proj, dtype=np.float32).reshape(KO, 128, C).transpose(1, 0, 2)
    ).astype(bf)
    b_attn = np.asarray(b_attn, dtype=np.float32)
    bqk = np.ascontiguousarray(b_attn[: 2 * C].reshape(16, 128).T)
    bv = np.ascontiguousarray(b_attn[2 * C :].reshape(1, C))
    bpj = np.ascontiguousarray(np.asarray(b_proj, dtype=np.float32).reshape(1, C))
    in_maps = [
        {
            "xT": np.ascontiguousarray(xT[i * BL : (i + 1) * BL]),
            "wa": wa,
            "wp": wp,
            "bqk": bqk,
            "bv": bv,
            "bpj": bpj,
        }
        for i in range(N_CORES)
    ]
    res = run_bass_kernel_spmd(nc, in_maps, core_ids=list(range(N_CORES)), trace=_trace)
    out = np.concatenate([res.results[i]["out"] for i in range(N_CORES)], axis=0)
    if _trace:
        kernel.last_results = res
    return out


# revision 15
# speedup vs baseline: 1.0057x; 1.0057x over previous
"""Causal self-attention Bass/Tile kernel for TRN2, data-parallel over 8 NeuronCores.

Shapes (hardcoded): x [16, 1024, 1024] f32, W_attn [1024, 3072], b_attn [3072],
W_proj [1024, 1024], b_proj [1024].  16 heads, head dim 64.
Each core processes 2 batch elements end-to-end; no collectives.

Design notes (v2):
  - x is uploaded pre-transposed per batch ([C, T] tiles) so no PE transposes.
  - All weights resident in SBUF, loaded once (W_attn 48KB/part, W_proj 16KB/part).
  - q^T,k^T per head pair via (W tile).T @ x^T; v natural via (x^T tile).T @ W_v.
  - scores^T = k^T.T @ q^T, K=64 row-packed pairs (tile_position) run concurrently.
  - AV: vext.T @ P^T col-packed with a ones[128,64] stationary at tile_position
    (0,64) so PSUM partitions 64..127 accumulate the softmax denominators;
    normalization = reciprocal + fused multiply-evict, no partition broadcast.
  - PE warm-up matmuls during the initial DMA wait (HAM clock gate).
  - qk of the next head pair / v / proj matmuls are interleaved as filler between
    score pairs to cover exp latency (PE queue is strict FIFO).
"""
import sys

sys.path.insert(0, "/opt/trn_rl_repo")

from contextlib import ExitStack

import numpy as np

import concourse.bass as bass
import concourse.mybir as mybir
import concourse.tile as tile
from concourse import bacc
from concourse.bass_utils import run_bass_kernel_spmd
from concourse.masks import make_upper_triangular

F32 = mybir.dt.float32
BF16 = mybir.dt.bfloat16
EXP = mybir.ActivationFunctionType.Exp

N_CORES = 8
B, T, C = 16, 1024, 1024
H, DH = 16, 64
BL = B // N_CORES          # batches per core
TT = T // 128              # token tiles (8)
KO = C // 128              # contraction chunks (8)
NQ = T // 512              # 512-wide token chunks (2)
SCALE = 1.0 / 8.0          # 1/sqrt(64)


def _emit(nc, tc, xT_d, wa_d, wp_d, bqk_d, bv_d, bpj_d, out_d, DBG=None):
    with ExitStack() as ctx:
        const = ctx.enter_context(tc.tile_pool(name="const", bufs=1))
        w_pool = ctx.enter_context(tc.tile_pool(name="w", bufs=1))
        xT_pool = ctx.enter_context(tc.tile_pool(name="xT", bufs=2))
        vext_pool = ctx.enter_context(tc.tile_pool(name="vext", bufs=2))
        qk_pool = ctx.enter_context(tc.tile_pool(name="qk", bufs=3))
        pt_pool = ctx.enter_context(tc.tile_pool(name="pt", bufs=6))
        yT_pool = ctx.enter_context(tc.tile_pool(name="yT", bufs=2))
        rec_pool = ctx.enter_context(tc.tile_pool(name="rec", bufs=2))
        out_pool = ctx.enter_context(tc.tile_pool(name="out", bufs=4))
        psQ = ctx.enter_context(tc.tile_pool(name="psQ", bufs=2, space="PSUM"))
        psS = ctx.enter_context(tc.tile_pool(name="psS", bufs=2, space="PSUM"))
        psAV = ctx.enter_context(tc.tile_pool(name="psAV", bufs=1, space="PSUM"))

        # ---- constants ----
        zmask = const.tile([128, 128], F32)
        make_upper_triangular(nc, zmask, val=1.0, diag=True)
        ones64 = const.tile([128, 64], BF16)
        nc.vector.memset(ones64, 1.0)
        warm = const.tile([128, 512], BF16)
        nc.vector.memset(warm, 0.0)
        b_qk = const.tile([128, 16], F32)
        nc.sync.dma_start(b_qk, bqk_d)
        bv_b = const.tile([128, C], F32)
        nc.sync.dma_start(bv_b, bv_d[0:1, :].to_broadcast((128, C)))
        bpj_b = const.tile([128, C], F32)
        nc.sync.dma_start(bpj_b, bpj_d[0:1, :].to_broadcast((128, C)))

        # ---- DMAs ordered so v(0) can start ASAP: xT0, W_v, then the rest ----
        wa_sb = w_pool.tile([128, KO, 3 * C], BF16, tag="wa", name="wa")
        wp_sb = w_pool.tile([128, KO, C], BF16, tag="wp", name="wp")
        xT = {}
        for b in range(BL):
            xT[b] = xT_pool.tile([128, KO, T], BF16, tag="xT", name=f"xT{b}")
        for k in range(KO):
            nc.sync.dma_start(xT[0][:, k, :], xT_d[0, k, :, :])
        for k in range(KO):  # v columns of W_attn, split for queue parallelism
            for nn in range(NQ):
                nc.sync.dma_start(
                    wa_sb[:, k, 2 * C + nn * 512 : 2 * C + (nn + 1) * 512],
                    wa_d[:, k, 2 * C + nn * 512 : 2 * C + (nn + 1) * 512],
                )
        for k in range(KO):
            nc.sync.dma_start(wa_sb[:, k, 0 : 2 * C], wa_d[:, k, 0 : 2 * C])
        for k in range(KO):
            nc.sync.dma_start(xT[1][:, k, :], xT_d[1, k, :, :])
        for k in range(KO):
            nc.sync.dma_start(wp_sb[:, k, :], wp_d[:, k, :])

        # ---- PE warm-up during DMA wait (HAM clock gate) ----
        for i in range(24):
            wps = psQ.tile([128, 512], F32, tag="q", name=f"warm{i}")
            nc.tensor.matmul(wps, warm[:, 0:128], warm, start=True, stop=True)

        vext = {}
        yT = {}
        qk = {}

        # ---------- thunk builders (each thunk = one PE matmul + attached evicts) ----------

        def v_thunks(b):
            """v for all heads of batch b -> vext[b]; 128 mms in 16 groups."""
            vext[b] = vext_pool.tile([128, TT, H, DH], BF16, tag="vext", name=f"vext{b}")
            thunks = []
            for nn in range(NQ):
                for m in range(TT):
                    ps = psQ.tile([128, 512], F32, tag="q", name=f"vps{b}_{nn}_{m}")
                    for k in range(KO):
                        def th(nn=nn, m=m, ps=ps, k=k):
                            nc.tensor.matmul(
                                ps,
                                xT[b][:, k, m * 128 : (m + 1) * 128],
                                wa_sb[:, k, 2 * C + nn * 512 : 2 * C + (nn + 1) * 512],
                                start=(k == 0),
                                stop=(k == KO - 1),
                            )
                            if k == KO - 1:
                                nc.vector.tensor_add(
                                    vext[b][:, m, nn * 8 : (nn + 1) * 8, :],
                                    ps.rearrange("p (h d) -> p h d", d=DH),
                                    bv_b[:, nn * 512 : (nn + 1) * 512].rearrange(
                                        "p (h d) -> p h d", d=DH
                                    ),
                                )
                        thunks.append(th)
            return thunks

        def qk_thunks(b, hp):
            """q^T,k^T for head pair hp of batch b; 32 mms in 4 groups."""
            qk[(b, hp)] = qk_pool.tile([128, 2, T], BF16, tag="qk", name=f"qk{b}_{hp}")
            thunks = []
            for which, mt in ((0, hp), (1, 8 + hp)):
                for nn in range(NQ):
                    ps = psQ.tile([128, 512], F32, tag="q", name=f"qkps{b}_{mt}_{nn}")
                    for k in range(KO):
                        def th(which=which, mt=mt, nn=nn, ps=ps, k=k):
                            nc.tensor.matmul(
                                ps,
                                wa_sb[:, k, mt * 128 : (mt + 1) * 128],
                                xT[b][:, k, nn * 512 : (nn + 1) * 512],
                                start=(k == 0),
                                stop=(k == KO - 1),
                            )
                            if k == KO - 1:
                                # evict on ScalarE (per-partition bias) to keep DVE free
                                nc.scalar.add(
                                    qk[(b, hp)][:, which, nn * 512 : (nn + 1) * 512],
                                    ps,
                                    b_qk[:, mt : mt + 1],
                                )
                        thunks.append(th)
            return thunks

        def proj_thunks(b):
            """out = y @ W_proj + b_proj for batch b; 128 mms in 16 groups."""
            thunks = []
            for nn in range(NQ):
                for m in range(TT):
                    ps = psQ.tile([128, 512], F32, tag="q", name=f"pps{b}_{nn}_{m}")
                    for k in range(KO):
                        def th(nn=nn, m=m, ps=ps, k=k):
                            nc.tensor.matmul(
                                ps,
                                yT[b][:, k, m * 128 : (m + 1) * 128],
                                wp_sb[:, k, nn * 512 : (nn + 1) * 512],
                                start=(k == 0),
                                stop=(k == KO - 1),
                            )
                            if k == KO - 1:
                                osb = out_pool.tile(
                                    [128, 512], F32, tag="o", name=f"os{b}_{nn}_{m}"
                                )
                                nc.vector.tensor_add(
                                    osb, ps, bpj_b[:, nn * 512 : (nn + 1) * 512]
                                )
                                nc.sync.dma_start(
                                    out_d[
                                        b,
                                        m * 128 : (m + 1) * 128,
                                        nn * 512 : (nn + 1) * 512,
                                    ],
                                    osb,
                                )
                        thunks.append(th)
            return thunks

        # ---------- attention inner loop for one head pair ----------

        def hp_loop(b, hp, fill):
            qk_t = qk[(b, hp)]
            n_slots = 12
            per_slot = max(1, -(-len(fill) // n_slots))
            fi = 0

            def emit_fill(n):
                nonlocal fi
                for _ in range(n):
                    if fi < len(fill):
                        fill[fi]()
                        fi += 1

            for qc in range(NQ):
                nkt = 4 * qc + 4
                Y0 = psAV.tile([128, 512], F32, tag="y0", name=f"y0_{b}_{hp}_{qc}")
                Y1 = psAV.tile([128, 512], F32, tag="y1", name=f"y1_{b}_{hp}_{qc}")
                D0 = psAV.tile([128, 512], F32, tag="d0", name=f"d0_{b}_{hp}_{qc}")
                D1 = psAV.tile([128, 512], F32, tag="d1", name=f"d1_{b}_{hp}_{qc}")
                h0, h1 = 2 * hp, 2 * hp + 1

                def av_slots(kt, pts, last):
                    # one PSUM bank per array position: concurrent pairs always
                    # target different banks (same-bank concurrent drain corrupts)
                    pst = 0 if kt - 4 * qc < 0 else (kt - 4 * qc) * 128
                    pt0, pt1 = pts
                    first = kt == 0
                    # slot A: V(h0)@(0,0)->Y0[0:64]  ||  ones@(0,64)->D1[64:128]
                    nc.tensor.matmul(
                        Y0[0:64, pst:512],
                        vext[b][:, kt, h0, :],
                        pt0[:, pst:512],
                        start=first,
                        stop=last,
                        tile_position=(0, 0),
                    )
                    nc.tensor.matmul(
                        D1[64:128, pst:512],
                        ones64,
                        pt1[:, pst:512],
                        start=first,
                        stop=last,
                        tile_position=(0, 64),
                    )
                    # slot B: ones@(0,0)->D0[0:64]  ||  V(h1)@(0,64)->Y1[64:128]
                    nc.tensor.matmul(
                        D0[0:64, pst:512],
                        ones64,
                        pt0[:, pst:512],
                        start=first,
                        stop=last,
                        tile_position=(0, 0),
                    )
                    nc.tensor.matmul(
                        Y1[64:128, pst:512],
                        vext[b][:, kt, h1, :],
                        pt1[:, pst:512],
                        start=first,
                        stop=last,
                        tile_position=(0, 64),
                    )

                prev = None
                for kt in range(nkt):
                    j = kt - 4 * qc
                    st = 0 if j < 0 else j * 128
                    # score pair (row-packed, concurrent, different banks)
                    sps = {}
                    for h2 in range(2):
                        sp = psS.tile(
                            [128, 512], F32, tag="sc", name=f"sc{b}_{hp}_{qc}_{kt}_{h2}"
                        )
                        nc.tensor.matmul(
                            sp[:, st:512],
                            qk_t[64 * h2 : 64 * h2 + 64, 1, kt * 128 : (kt + 1) * 128],
                            qk_t[
                                64 * h2 : 64 * h2 + 64,
                                0,
                                qc * 512 + st : (qc + 1) * 512,
                            ],
                            start=True,
                            stop=True,
                            tile_position=(64 * h2, 0),
                        )
                        sps[h2] = sp
                    emit_fill(per_slot)
                    # AV for the previous kt (exp latency covered by filler above)
                    if prev is not None:
                        av_slots(prev[0], prev[1], last=False)
                    # exp (+ causal mask on the diagonal block)
                    cpts = []
                    for h2 in range(2):
                        pt = pt_pool.tile(
                            [128, 512], BF16, tag="pt", name=f"pt{b}_{hp}_{qc}_{kt}_{h2}"
                        )
                        nc.scalar.activation(
                            pt[:, st:512], sps[h2][:, st:512], EXP, scale=SCALE
                        )
                        if j >= 0:
                            nc.vector.tensor_mul(
                                pt[:, st : st + 128], pt[:, st : st + 128], zmask
                            )
                        cpts.append(pt)
                    prev = (kt, cpts)
                # AV for the last kt
                av_slots(prev[0], prev[1], last=True)
                # normalize + evict into yT (partition-aligned per head)
                rec = rec_pool.tile([128, 512], F32, tag="rec", name=f"rec{b}_{hp}_{qc}")
                nc.vector.reciprocal(rec[0:64, :], D0[0:64, :])
                nc.vector.reciprocal(rec[64:128, :], D1[64:128, :])
                nc.vector.tensor_mul(
                    yT[b][0:64, hp, qc * 512 : (qc + 1) * 512], Y0[0:64, :], rec[0:64, :]
                )
                nc.vector.tensor_mul(
                    yT[b][64:128, hp, qc * 512 : (qc + 1) * 512],
                    Y1[64:128, :],
                    rec[64:128, :],
                )
            # drain remaining filler
            emit_fill(len(fill))
            if DBG is not None and b == 0:
                nc.sync.dma_start(DBG["qk"][hp], qk_t)

        # ---------- schedule ----------
        for b in range(BL):
            yT[b] = yT_pool.tile([128, KO, T], BF16, tag="yT", name=f"yT{b}")

        for th in v_thunks(0):
            th()
        for th in qk_thunks(0, 0):
            th()

        v1 = v_thunks(1)
        for hp in range(TT):
            fill = []
            if hp < 7:
                fill += qk_thunks(0, hp + 1)
            if hp >= 4:
                fill += v1[(hp - 4) * 32 : (hp - 3) * 32]
            if hp == 7:
                fill += qk_thunks(1, 0)
            hp_loop(0, hp, fill)

        p0 = proj_thunks(0)
        for hp in range(TT):
            fill = []
            if hp < 7:
                fill += qk_thunks(1, hp + 1)
            fill += p0[hp * 16 : (hp + 1) * 16]
            hp_loop(1, hp, fill)

        for th in proj_thunks(1):
            th()

        if DBG is not None:
            nc.sync.dma_start(DBG["vext"], vext[0])
            nc.sync.dma_start(DBG["yT"], yT[0])


_CACHE = {}


def _build(debug=False):
    key = "nc_dbg" if debug else "nc"
    if key in _CACHE:
        return _CACHE[key]
    nc = bacc.Bacc("TRN2", target_bir_lowering=False, debug=False)
    xT_d = nc.dram_tensor("xT", [BL, KO, 128, T], BF16, kind="ExternalInput").ap()
    wa_d = nc.dram_tensor("wa", [128, KO, 3 * C], BF16, kind="ExternalInput").ap()
    wp_d = nc.dram_tensor("wp", [128, KO, C], BF16, kind="ExternalInput").ap()
    bqk_d = nc.dram_tensor("bqk", [128, 16], F32, kind="ExternalInput").ap()
    bv_d = nc.dram_tensor("bv", [1, C], F32, kind="ExternalInput").ap()
    bpj_d = nc.dram_tensor("bpj", [1, C], F32, kind="ExternalInput").ap()
    out_d = nc.dram_tensor("out", [BL, T, C], F32, kind="ExternalOutput").ap()
    DBG = None
    if debug:
        DBG = {
            "qk": [
                nc.dram_tensor(f"dqk{hp}", [128, 2, T], BF16, kind="ExternalOutput").ap()
                for hp in range(KO)
            ],
            "vext": nc.dram_tensor("dvext", [128, TT, H, DH], BF16, kind="ExternalOutput").ap(),
            "yT": nc.dram_tensor("dyT", [128, KO, T], BF16, kind="ExternalOutput").ap(),
        }
    with tile.TileContext(nc) as tc:
        _emit(nc, tc, xT_d, wa_d, wp_d, bqk_d, bv_d, bpj_d, out_d, DBG=DBG)
    nc.compile()
    _CACHE[key] = nc
    return nc


def kernel(x, W_attn, b_attn, W_proj, b_proj, _trace=False):
    nc = _build()
    import ml_dtypes

    bf = ml_dtypes.bfloat16
    x = np.asarray(x, dtype=np.float32)
    # per-batch transpose to [C, T], tiled [KO, 128, T]
    xT = np.ascontiguousarray(np.transpose(x, (0, 2, 1))).reshape(B, KO, 128, T).astype(bf)
    wa = np.ascontiguousarray(
        np.asarray(W_attn, dtype=np.float32).reshape(KO, 128, 3 * C).transpose(1, 0, 2)
    ).astype(bf)
    wp = np.ascontiguousarray(
        np.asarray(W_proj, dtype=np.float32).reshape(KO, 128, C).transpose(1, 0, 2)
    ).astype(bf)
    b_attn = np.asarray(b_attn, dtype=np.float32)
    bqk = np.ascontiguousarray(b_attn[: 2 * C].reshape(16, 128).T)
    bv = np.ascontiguousarray(b_attn[2 * C :].reshape(1, C))
    bpj = np.ascontiguousarray(np.asarray(b_proj, dtype=np.float32).reshape(1, C))
    in_maps = [
        {
            "xT": np.ascontiguousarray(xT[i * BL : (i + 1) * BL]),
            "wa": wa,
            "wp": wp,
            "bqk": bqk,
            "bv": bv,
            "bpj": bpj,
        }
        for i in range(N_CORES)
    ]
    res = run_bass_kernel_spmd(nc, in_maps, core_ids=list(range(N_CORES)), trace=_trace)
    out = np.concatenate([res.results[i]["out"] for i in range(N_CORES)], axis=0)
    if _trace:
        kernel.last_results = res
    return out


# revision 19
# speedup vs baseline: 1.4617x; 1.4534x over previous
"""Causal self-attention Bass/Tile kernel for TRN2, data-parallel over 8 NeuronCores.

Shapes (hardcoded): x [16, 1024, 1024] f32, W_attn [1024, 3072], b_attn [3072],
W_proj [1024, 1024], b_proj [1024].  16 heads, head dim 64.
Each core processes 2 batch elements end-to-end; no collectives.

Design notes (v2):
  - x is uploaded pre-transposed per batch ([C, T] tiles) so no PE transposes.
  - All weights resident in SBUF, loaded once (W_attn 48KB/part, W_proj 16KB/part).
  - q^T,k^T per head pair via (W tile).T @ x^T; v natural via (x^T tile).T @ W_v.
  - scores^T = k^T.T @ q^T, K=64 row-packed pairs (tile_position) run concurrently.
  - AV: vext.T @ P^T col-packed with a ones[128,64] stationary at tile_position
    (0,64) so PSUM partitions 64..127 accumulate the softmax denominators;
    normalization = reciprocal + fused multiply-evict, no partition broadcast.
  - PE warm-up matmuls during the initial DMA wait (HAM clock gate).
  - qk of the next head pair / v / proj matmuls are interleaved as filler between
    score pairs to cover exp latency (PE queue is strict FIFO).
"""
import sys

sys.path.insert(0, "/opt/trn_rl_repo")

from contextlib import ExitStack

import numpy as np

import concourse.bass as bass
import concourse.mybir as mybir
import concourse.tile as tile
from concourse import bacc
from concourse.bass_utils import run_bass_kernel_spmd
from concourse.masks import make_upper_triangular

F32 = mybir.dt.float32
BF16 = mybir.dt.bfloat16
EXP = mybir.ActivationFunctionType.Exp

N_CORES = 8
B, T, C = 16, 1024, 1024
H, DH = 16, 64
BL = B // N_CORES          # batches per core
TT = T // 128              # token tiles (8)
KO = C // 128              # contraction chunks (8)
NQ = T // 512              # 512-wide token chunks (2)
SCALE = 1.0 / 8.0          # 1/sqrt(64)


def _emit(nc, tc, xT_d, wa_d, wp_d, bqk_d, bv_d, bpj_d, out_d, DBG=None):
    with ExitStack() as ctx:
        const = ctx.enter_context(tc.tile_pool(name="const", bufs=1))
        w_pool = ctx.enter_context(tc.tile_pool(name="w", bufs=1))
        xT_pool = ctx.enter_context(tc.tile_pool(name="xT", bufs=2))
        vext_pool = ctx.enter_context(tc.tile_pool(name="vext", bufs=2))
        qk_pool = ctx.enter_context(tc.tile_pool(name="qk", bufs=3))
        pt_pool = ctx.enter_context(tc.tile_pool(name="pt", bufs=6))
        yT_pool = ctx.enter_context(tc.tile_pool(name="yT", bufs=2))
        rec_pool = ctx.enter_context(tc.tile_pool(name="rec", bufs=1))
        out_pool = ctx.enter_context(tc.tile_pool(name="out", bufs=4))
        psQ = ctx.enter_context(tc.tile_pool(name="psQ", bufs=2, space="PSUM"))
        psS = ctx.enter_context(tc.tile_pool(name="psS", bufs=2, space="PSUM"))
        psAV = ctx.enter_context(tc.tile_pool(name="psAV", bufs=1, space="PSUM"))

        # ---- constants ----
        zmask = const.tile([128, 128], F32)
        make_upper_triangular(nc, zmask, val=1.0, diag=True)
        ones64 = const.tile([128, 64], BF16)
        nc.vector.memset(ones64, 1.0)
        warm = const.tile([128, 512], BF16)
        nc.vector.memset(warm, 0.0)
        b_qk = const.tile([128, 16], F32)
        nc.sync.dma_start(b_qk, bqk_d)
        bv_b = const.tile([128, C], F32)
        nc.sync.dma_start(bv_b, bv_d[0:1, :].to_broadcast((128, C)))
        bpj_b = const.tile([128, C], F32)
        nc.sync.dma_start(bpj_b, bpj_d[0:1, :].to_broadcast((128, C)))

        # ---- DMAs ordered so v(0) can start ASAP: xT0, W_v, then the rest ----
        wa_sb = w_pool.tile([128, KO, 3 * C], BF16, tag="wa", name="wa")
        wp_sb = w_pool.tile([128, KO, C], BF16, tag="wp", name="wp")
        xT = {}
        for b in range(BL):
            xT[b] = xT_pool.tile([128, KO, T], BF16, tag="xT", name=f"xT{b}")
        for k in range(KO):
            nc.sync.dma_start(xT[0][:, k, :], xT_d[0, k, :, :])
        for k in range(KO):  # v columns of W_attn, split for queue parallelism
            for nn in range(NQ):
                nc.sync.dma_start(
                    wa_sb[:, k, 2 * C + nn * 512 : 2 * C + (nn + 1) * 512],
                    wa_d[:, k, 2 * C + nn * 512 : 2 * C + (nn + 1) * 512],
                )
        for k in range(KO):
            nc.sync.dma_start(wa_sb[:, k, 0 : 2 * C], wa_d[:, k, 0 : 2 * C])
        for k in range(KO):
            nc.sync.dma_start(xT[1][:, k, :], xT_d[1, k, :, :])
        for k in range(KO):
            nc.sync.dma_start(wp_sb[:, k, :], wp_d[:, k, :])

        # ---- PE warm-up during DMA wait (HAM clock gate) ----
        for i in range(24):
            wps = psQ.tile([128, 512], F32, tag="q", name=f"warm{i}")
            nc.tensor.matmul(wps, warm[:, 0:128], warm, start=True, stop=True)

        vext = {}
        yT = {}
        qk = {}

        # ---------- thunk builders (each thunk = one PE matmul + attached evicts) ----------

        def v_thunks(b):
            """v for all heads of batch b -> vext[b]; 128 mms in 16 groups."""
            vext[b] = vext_pool.tile([128, TT, H, DH], BF16, tag="vext", name=f"vext{b}")
            thunks = []
            for nn in range(NQ):
                for m in range(TT):
                    ps = psQ.tile([128, 512], F32, tag="q", name=f"vps{b}_{nn}_{m}")
                    for k in range(KO):
                        def th(nn=nn, m=m, ps=ps, k=k):
                            nc.tensor.matmul(
                                ps,
                                xT[b][:, k, m * 128 : (m + 1) * 128],
                                wa_sb[:, k, 2 * C + nn * 512 : 2 * C + (nn + 1) * 512],
                                start=(k == 0),
                                stop=(k == KO - 1),
                            )
                            if k == KO - 1:
                                nc.vector.tensor_add(
                                    vext[b][:, m, nn * 8 : (nn + 1) * 8, :],
                                    ps.rearrange("p (h d) -> p h d", d=DH),
                                    bv_b[:, nn * 512 : (nn + 1) * 512].rearrange(
                                        "p (h d) -> p h d", d=DH
                                    ),
                                )
                        thunks.append(th)
            return thunks

        def qk_thunks(b, hp):
            """q^T,k^T for head pair hp of batch b; 32 mms in 4 groups."""
            qk[(b, hp)] = qk_pool.tile([128, 2, T], BF16, tag="qk", name=f"qk{b}_{hp}")
            thunks = []
            for which, mt in ((0, hp), (1, 8 + hp)):
                for nn in range(NQ):
                    ps = psQ.tile([128, 512], F32, tag="q", name=f"qkps{b}_{mt}_{nn}")
                    for k in range(KO):
                        def th(which=which, mt=mt, nn=nn, ps=ps, k=k):
                            nc.tensor.matmul(
                                ps,
                                wa_sb[:, k, mt * 128 : (mt + 1) * 128],
                                xT[b][:, k, nn * 512 : (nn + 1) * 512],
                                start=(k == 0),
                                stop=(k == KO - 1),
                            )
                            if k == KO - 1:
                                # evict on ScalarE (per-partition bias) to keep DVE free
                                nc.scalar.add(
                                    qk[(b, hp)][:, which, nn * 512 : (nn + 1) * 512],
                                    ps,
                                    b_qk[:, mt : mt + 1],
                                )
                        thunks.append(th)
            return thunks

        def proj_thunks(b):
            """out = y @ W_proj + b_proj for batch b; 128 mms in 16 groups."""
            thunks = []
            for nn in range(NQ):
                for m in range(TT):
                    ps = psQ.tile([128, 512], F32, tag="q", name=f"pps{b}_{nn}_{m}")
                    for k in range(KO):
                        def th(nn=nn, m=m, ps=ps, k=k):
                            nc.tensor.matmul(
                                ps,
                                yT[b][:, k, m * 128 : (m + 1) * 128],
                                wp_sb[:, k, nn * 512 : (nn + 1) * 512],
                                start=(k == 0),
                                stop=(k == KO - 1),
                            )
                            if k == KO - 1:
                                osb = out_pool.tile(
                                    [128, 512], F32, tag="o", name=f"os{b}_{nn}_{m}"
                                )
                                nc.vector.tensor_add(
                                    osb, ps, bpj_b[:, nn * 512 : (nn + 1) * 512]
                                )
                                nc.sync.dma_start(
                                    out_d[
                                        b,
                                        m * 128 : (m + 1) * 128,
                                        nn * 512 : (nn + 1) * 512,
                                    ],
                                    osb,
                                )
                        thunks.append(th)
            return thunks

        # ---------- attention inner loop for one head pair ----------

        def hp_loop(b, hp, fill):
            qk_t = qk[(b, hp)]
            n_slots = 12
            per_slot = max(1, -(-len(fill) // n_slots))
            fi = 0

            def emit_fill(n):
                nonlocal fi
                for _ in range(n):
                    if fi < len(fill):
                        fill[fi]()
                        fi += 1

            for qc in range(NQ):
                nkt = 4 * qc + 4
                Y0 = psAV.tile([128, 512], F32, tag="y0", name=f"y0_{b}_{hp}_{qc}")
                Y1 = psAV.tile([128, 512], F32, tag="y1", name=f"y1_{b}_{hp}_{qc}")
                D0 = psAV.tile([128, 512], F32, tag="d0", name=f"d0_{b}_{hp}_{qc}")
                D1 = psAV.tile([128, 512], F32, tag="d1", name=f"d1_{b}_{hp}_{qc}")
                h0, h1 = 2 * hp, 2 * hp + 1

                def av_slots(kt, pts, last):
                    # one PSUM bank per array position: concurrent pairs always
                    # target different banks (same-bank concurrent drain corrupts)
                    pst = 0 if kt - 4 * qc < 0 else (kt - 4 * qc) * 128
                    pt0, pt1 = pts
                    first = kt == 0
                    # slot A: V(h0)@(0,0)->Y0[0:64]  ||  ones@(0,64)->D1[64:128]
                    nc.tensor.matmul(
                        Y0[0:64, pst:512],
                        vext[b][:, kt, h0, :],
                        pt0[:, pst:512],
                        start=first,
                        stop=last,
                        tile_position=(0, 0),
                    )
                    nc.tensor.matmul(
                        D1[64:128, pst:512],
                        ones64,
                        pt1[:, pst:512],
                        start=first,
                        stop=last,
                        tile_position=(0, 64),
                    )
                    # slot B: ones@(0,0)->D0[0:64]  ||  V(h1)@(0,64)->Y1[64:128]
                    nc.tensor.matmul(
                        D0[0:64, pst:512],
                        ones64,
                        pt0[:, pst:512],
                        start=first,
                        stop=last,
                        tile_position=(0, 0),
                    )
                    nc.tensor.matmul(
                        Y1[64:128, pst:512],
                        vext[b][:, kt, h1, :],
                        pt1[:, pst:512],
                        start=first,
                        stop=last,
                        tile_position=(0, 64),
                    )

                prev = None
                for kt in range(nkt):
                    j = kt - 4 * qc
                    st = 0 if j < 0 else j * 128
                    # score pair (row-packed, concurrent, different banks)
                    sps = {}
                    for h2 in range(2):
                        sp = psS.tile(
                            [128, 512], F32, tag="sc", name=f"sc{b}_{hp}_{qc}_{kt}_{h2}"
                        )
                        nc.tensor.matmul(
                            sp[:, st:512],
                            qk_t[64 * h2 : 64 * h2 + 64, 1, kt * 128 : (kt + 1) * 128],
                            qk_t[
                                64 * h2 : 64 * h2 + 64,
                                0,
                                qc * 512 + st : (qc + 1) * 512,
                            ],
                            start=True,
                            stop=True,
                            tile_position=(64 * h2, 0),
                        )
                        sps[h2] = sp
                    emit_fill(per_slot)
                    # AV for the previous kt (exp latency covered by filler above)
                    if prev is not None:
                        av_slots(prev[0], prev[1], last=False)
                    # exp (+ causal mask on the diagonal block)
                    cpts = []
                    for h2 in range(2):
                        pt = pt_pool.tile(
                            [128, 512], BF16, tag="pt", name=f"pt{b}_{hp}_{qc}_{kt}_{h2}"
                        )
                        nc.scalar.activation(
                            pt[:, st:512], sps[h2][:, st:512], EXP, scale=SCALE
                        )
                        if j >= 0:
                            nc.vector.tensor_mul(
                                pt[:, st : st + 128], pt[:, st : st + 128], zmask
                            )
                        cpts.append(pt)
                    prev = (kt, cpts)
                # AV for the last kt
                av_slots(prev[0], prev[1], last=True)
                # normalize + evict into yT (partition-aligned per head)
                # reciprocal_approx_fast requires partition offset 0 for both
                # operands (custom-DVE uop limitation) — stage D1 down to 0 first
                rec = rec_pool.tile([64, 3, 512], F32, tag="rec", name=f"rec{b}_{hp}_{qc}")
                nc.vector.reciprocal_approx_fast(rec[:, 0, :], D0[0:64, :])
                nc.vector.tensor_mul(
                    yT[b][0:64, hp, qc * 512 : (qc + 1) * 512],
                    Y0[0:64, :],
                    rec[:, 0, :],
                )
                nc.vector.tensor_copy(rec[:, 1, :], D1[64:128, :])
                nc.vector.reciprocal_approx_fast(rec[:, 2, :], rec[:, 1, :])
                nc.vector.tensor_mul(
                    yT[b][64:128, hp, qc * 512 : (qc + 1) * 512],
                    Y1[64:128, :],
                    rec[:, 2, :],
                )
            # drain remaining filler
            emit_fill(len(fill))
            if DBG is not None and b == 0:
                nc.sync.dma_start(DBG["qk"][hp], qk_t)

        # ---------- schedule ----------
        for b in range(BL):
            yT[b] = yT_pool.tile([128, KO, T], BF16, tag="yT", name=f"yT{b}")

        for th in v_thunks(0):
            th()
        for th in qk_thunks(0, 0):
            th()

        v1 = v_thunks(1)
        for hp in range(TT):
            fill = []
            if hp < 7:
                fill += qk_thunks(0, hp + 1)
            if hp >= 4:
                fill += v1[(hp - 4) * 32 : (hp - 3) * 32]
            if hp == 7:
                fill += qk_thunks(1, 0)
            hp_loop(0, hp, fill)

        p0 = proj_thunks(0)
        for hp in range(TT):
            fill = []
            if hp < 7:
                fill += qk_thunks(1, hp + 1)
            fill += p0[hp * 16 : (hp + 1) * 16]
            hp_loop(1, hp, fill)

        for th in proj_thunks(1):
            th()

        if DBG is not None:
            nc.sync.dma_start(DBG["vext"], vext[0])
            nc.sync.dma_start(DBG["yT"], yT[0])


_CACHE = {}


def _build(debug=False):
    key = "nc_dbg" if debug else "nc"
    if key in _CACHE:
        return _CACHE[key]
    nc = bacc.Bacc("TRN2", target_bir_lowering=False, debug=False)
    xT_d = nc.dram_tensor("xT", [BL, KO, 128, T], BF16, kind="ExternalInput").ap()
    wa_d = nc.dram_tensor("wa", [128, KO, 3 * C], BF16, kind="ExternalInput").ap()
    wp_d = nc.dram_tensor("wp", [128, KO, C], BF16, kind="ExternalInput").ap()
    bqk_d = nc.dram_tensor("bqk", [128, 16], F32, kind="ExternalInput").ap()
    bv_d = nc.dram_tensor("bv", [1, C], F32, kind="ExternalInput").ap()
    bpj_d = nc.dram_tensor("bpj", [1, C], F32, kind="ExternalInput").ap()
    out_d = nc.dram_tensor("out", [BL, T, C], F32, kind="ExternalOutput").ap()
    DBG = None
    if debug:
        DBG = {
            "qk": [
                nc.dram_tensor(f"dqk{hp}", [128, 2, T], BF16, kind="ExternalOutput").ap()
                for hp in range(KO)
            ],
            "vext": nc.dram_tensor("dvext", [128, TT, H, DH], BF16, kind="ExternalOutput").ap(),
            "yT": nc.dram_tensor("dyT", [128, KO, T], BF16, kind="ExternalOutput").ap(),
        }
    with tile.TileContext(nc) as tc:
        _emit(nc, tc, xT_d, wa_d, wp_d, bqk_d, bv_d, bpj_d, out_d, DBG=DBG)
    nc.compile()
    _CACHE[key] = nc
    return nc


def kernel(x, W_attn, b_attn, W_proj, b_proj, _trace=False):
    nc = _build()
    import ml_dtypes

    bf = ml_dtypes.bfloat16
    x = np.asarray(x, dtype=np.float32)
    # per-batch transpose to [C, T], tiled [KO, 128, T]
    xT = np.ascontiguousarray(np.transpose(x, (0, 2, 1))).reshape(B, KO, 128, T).astype(bf)
    wa = np.ascontiguousarray(
        np.asarray(W_attn, dtype=np.float32).reshape(KO, 128, 3 * C).transpose(1, 0, 2)
    ).astype(bf)
    wp = np.ascontiguousarray(
        np.asarray(W_proj, dtype=np.float32).reshape(KO, 128, C).transpose(1, 0, 2)
    ).astype(bf)
    b_attn = np.asarray(b_attn, dtype=np.float32)
    bqk = np.ascontiguousarray(b_attn[: 2 * C].reshape(16, 128).T)
    bv = np.ascontiguousarray(b_attn[2 * C :].reshape(1, C))
    bpj = np.ascontiguousarray(np.asarray(b_proj, dtype=np.float32).reshape(1, C))
    in_maps = [
        {
            "xT": np.ascontiguousarray(xT[i * BL : (i + 1) * BL]),
            "wa": wa,
            "wp": wp,
            "bqk": bqk,
            "bv": bv,
            "bpj": bpj,
        }
        for i in range(N_CORES)
    ]
    res = run_bass_kernel_spmd(nc, in_maps, core_ids=list(range(N_CORES)), trace=_trace)
    out = np.concatenate([res.results[i]["out"] for i in range(N_CORES)], axis=0)
    if _trace:
        kernel.last_results = res
    return out
